# revision 6
# baseline (speedup 1.0000x reference)
"""Causal attention (B=4, S=2048, D=1024) on 8 TRN2 NeuronCores.

Sharding: core c -> batch c//2, query-half c%2. Each core computes K/V for
all 2048 keys of its batch and attention for 1024 queries. Queries are
regrouped (host-side) into 4 groups of 256 pairing complementary causal
blocks, so one SPMD program with a fixed key-prefix schedule [4,8,12,16]
kblocks serves both halves; per-core causal structure lives in input data
(xqT column gather + qpos vector), never in program constants.

Math: scoresT[k,q] = KT^T QT accumulated over d in PSUM, probs =
exp(scoresT/32) (no max subtraction: logits ~ N(0,1)), causal mask applied
as a multiplicative (qpos >= kpos) keep-mask after exp, out = P^T V with
row-sums from a ones-column matmul, normalized at eviction.

Precision: Q/K/scores in float32r (tf32-class), V/probs bf16, fp32 accum.
"""

import numpy as np

import concourse.bass as bass
import concourse.mybir as mybir
import concourse.tile as tile
from concourse import bacc
from concourse.bass_utils import run_bass_kernel_spmd

B, S, D = 4, 2048, 1024
P = 128
NQ = S // 2               # queries per core
DT = D // P               # 8 d-tiles
KI = D // P               # 8 contraction tiles
NKB = S // P              # 16 key blocks
NG = 4                    # query groups per core
GQ = 256                  # queries per group
LKB = [4, 8, 12, 16]      # key-prefix (in kblocks) per group
MASK_START = [0, 4, 8, 12]  # first kblock needing the causal keep-mask

# per-core query block order (global block index within the batch)
QLIST = {
    0: [0, 2, 4, 6, 9, 11, 13, 15],
    1: [1, 3, 5, 7, 8, 10, 12, 14],
}

F32 = mybir.dt.float32
F32R = mybir.dt.float32r
BF16 = mybir.dt.bfloat16
AF = mybir.ActivationFunctionType

_NC_CACHE = []


def _build_nc():
    nc = bacc.Bacc("TRN2")
    xT = nc.dram_tensor("xT", [D, S], F32, kind="ExternalInput")
    xqT = nc.dram_tensor("xqT", [D, NQ], F32, kind="ExternalInput")
    wqT = nc.dram_tensor("wqT", [D, D], F32, kind="ExternalInput")
    wkT = nc.dram_tensor("wkT", [D, D], F32, kind="ExternalInput")
    wvT = nc.dram_tensor("wvT", [D, D], F32, kind="ExternalInput")
    qpos = nc.dram_tensor("qpos", [1, NQ], F32, kind="ExternalInput")
    out = nc.dram_tensor("out", [NQ, D], F32, kind="ExternalOutput")
    v_tmp = nc.dram_tensor("v_tmp", [NKB, P, D], BF16, kind="Internal")

    with tile.TileContext(nc) as tc:
        with tc.tile_pool(name="const", bufs=1) as const:
            ones_row = const.tile([1, P], F32, name="ones_row")
            nc.vector.memset(ones_row[:], 1.0)
            ones_col = const.tile([P, 1], BF16, name="ones_col")
            nc.vector.memset(ones_col[:], 1.0)
            kpos_i = const.tile([P, 1], mybir.dt.int32, name="kpos_i")
            nc.gpsimd.iota(kpos_i[:], pattern=[[0, 1]], base=0, channel_multiplier=1)
            kpos_f = const.tile([P, 1], F32, name="kpos_f")
            nc.vector.tensor_copy(kpos_f[:], kpos_i[:])
            qpos_sb = const.tile([1, NQ], F32, name="qpos_sb")
            nc.sync.dma_start(out=qpos_sb[:], in_=qpos[:])

            # ---- persistent Q^T / K^T ----
            with tc.tile_pool(name="acts", bufs=1) as acts:
                qt_sb = acts.tile([P, DT, NQ], F32R, name="qt_sb")
                kt_sb = acts.tile([P, DT, S], F32R, name="kt_sb")

                # ---- Q projection (before xT residency, to fit SBUF) ----
                with tc.tile_pool(name="xqr_p", bufs=1) as xqr_pool, \
                     tc.tile_pool(name="wq_p", bufs=2) as wq_pool, \
                     tc.tile_pool(name="pj_q", bufs=4, space="PSUM") as pj:
                    xqr = xqr_pool.tile([P, KI, NQ], F32R, name="xqr")
                    with tc.tile_pool(name="xq_st", bufs=2) as xqstage:
                        for ki in range(KI):
                            stg = xqstage.tile([P, NQ], F32, name=f"xqs{ki}",
                                               tag="xqs")
                            nc.sync.dma_start(
                                out=stg[:], in_=xqT[ki * P:(ki + 1) * P, :]
                            )
                            nc.vector.tensor_copy(xqr[:, ki, :], stg[:])
                    for dt in range(DT):
                        wq_st = wq_pool.tile([P, KI, P], F32, name=f"wqs{dt}",
                                             tag="wqs")
                        nc.sync.dma_start(
                            out=wq_st[:],
                            in_=wqT[:, dt * P:(dt + 1) * P].rearrange(
                                "(k p) o -> p k o", p=P
                            ),
                        )
                        wq_r = wq_pool.tile([P, KI, P], F32R, name=f"wqr{dt}",
                                            tag="wqr")
                        nc.vector.tensor_copy(wq_r[:], wq_st[:])
                        for qc in range(NQ // 512):
                            ps = pj.tile([P, 512], F32, name=f"psq{dt}_{qc}",
                                         tag="pjq")
                            for ki in range(KI):
                                nc.tensor.matmul(
                                    ps[:],
                                    wq_r[:, ki, :],
                                    xqr[:, ki, qc * 512:(qc + 1) * 512],
                                    start=(ki == 0),
                                    stop=(ki == KI - 1),
                                )
                            nc.scalar.copy(
                                qt_sb[:, dt, qc * 512:(qc + 1) * 512], ps[:]
                            )

                # ---- load + round xT, then K and V projections ----
                with tc.tile_pool(name="xtr_p", bufs=1) as xtr_pool:
                    xtr = xtr_pool.tile([P, KI, S], F32R, name="xtr")
                    with tc.tile_pool(name="x_st", bufs=2) as xstage:
                        for ki in range(KI):
                            stg = xstage.tile([P, S], F32, name=f"xs{ki}",
                                              tag="xs")
                            nc.sync.dma_start(
                                out=stg[:], in_=xT[ki * P:(ki + 1) * P, :]
                            )
                            nc.vector.tensor_copy(xtr[:, ki, :], stg[:])

                    with tc.tile_pool(name="wk_p", bufs=2) as wk_pool, \
                         tc.tile_pool(name="pj_k", bufs=4, space="PSUM") as pjk:
                        for dt in range(DT):
                            wk_st = wk_pool.tile([P, KI, P], F32, name=f"wks{dt}",
                                                 tag="wks")
                            nc.sync.dma_start(
                                out=wk_st[:],
                                in_=wkT[:, dt * P:(dt + 1) * P].rearrange(
                                    "(k p) o -> p k o", p=P
                                ),
                            )
                            wk_r = wk_pool.tile([P, KI, P], F32R, name=f"wkr{dt}",
                                                tag="wkr")
                            nc.vector.tensor_copy(wk_r[:], wk_st[:])
                            for kc in range(S // 512):
                                ps = pjk.tile([P, 512], F32, name=f"psk{dt}_{kc}",
                                              tag="pjk")
                                for ki in range(KI):
                                    nc.tensor.matmul(
                                        ps[:],
                                        wk_r[:, ki, :],
                                        xtr[:, ki, kc * 512:(kc + 1) * 512],
                                        start=(ki == 0),
                                        stop=(ki == KI - 1),
                                    )
                                nc.scalar.copy(
                                    kt_sb[:, dt, kc * 512:(kc + 1) * 512], ps[:]
                                )

                    # V: v[s, o] = sum_ki xT[ki,s]^T wvT[ki,o] -> DRAM (bf16)
                    with tc.tile_pool(name="wv_p", bufs=1) as wv_pool, \
                         tc.tile_pool(name="wv_s", bufs=2) as wv_stage, \
                         tc.tile_pool(name="v_ev", bufs=3) as vst_pool, \
                         tc.tile_pool(name="pj_v", bufs=4, space="PSUM") as pjv:
                        for oc in range(D // 512):
                            wv_r = wv_pool.tile([P, KI, 512], F32R,
                                                name=f"wvr{oc}", tag="wvr")
                            for ki in range(KI):
                                stg = wv_stage.tile([P, 512], F32,
                                                    name=f"wvs{oc}_{ki}", tag="wvs")
                                nc.sync.dma_start(
                                    out=stg[:],
                                    in_=wvT[ki * P:(ki + 1) * P,
                                            oc * 512:(oc + 1) * 512],
                                )
                                nc.vector.tensor_copy(wv_r[:, ki, :], stg[:])
                            for st_i in range(NKB):
                                ps = pjv.tile([P, 512], F32, name=f"psv{oc}_{st_i}",
                                              tag="pjv")
                                for ki in range(KI):
                                    nc.tensor.matmul(
                                        ps[:],
                                        xtr[:, ki, st_i * P:(st_i + 1) * P],
                                        wv_r[:, ki, :],
                                        start=(ki == 0),
                                        stop=(ki == KI - 1),
                                    )
                                vs = vst_pool.tile([P, 512], BF16,
                                                   name=f"vsb{oc}_{st_i}", tag="vsb")
                                nc.scalar.copy(vs[:], ps[:])
                                nc.sync.dma_start(
                                    out=v_tmp[st_i, :, oc * 512:(oc + 1) * 512],
                                    in_=vs[:],
                                )

                # ---- attention ----
                with tc.tile_pool(name="att", bufs=2) as att, \
                     tc.tile_pool(name="vin", bufs=3) as vin, \
                     tc.tile_pool(name="ptp", bufs=3) as ptp, \
                     tc.tile_pool(name="scp", bufs=2, space="PSUM") as scp, \
                     tc.tile_pool(name="avp", bufs=4, space="PSUM") as avp, \
                     tc.tile_pool(name="smp", bufs=2, space="PSUM") as smp, \
                     tc.tile_pool(name="evp", bufs=2) as ev:
                    for g in range(NG):
                        lkb = LKB[g]
                        # broadcast qpos slice to 128 partitions via ones matmul
                        ps_bc = scp.tile([P, GQ], F32, name=f"psbc{g}", tag="sc")
                        nc.tensor.matmul(
                            ps_bc[:],
                            ones_row[:],
                            qpos_sb[:, g * GQ:(g + 1) * GQ],
                            start=True,
                            stop=True,
                        )
                        qbc = att.tile([P, GQ], F32, name=f"qbc{g}", tag="qbc")
                        nc.vector.tensor_copy(qbc[:], ps_bc[:])

                        oa = [
                            avp.tile([P, 512], F32, name=f"oa{g}_{i}", tag="av")
                            for i in range(4)
                        ]
                        sm = [
                            smp.tile([P, 1], F32, name=f"sm{g}_{i}", tag="sm")
                            for i in range(2)
                        ]

                        for kb in range(lkb):
                            ps_s = scp.tile([P, GQ], F32, name=f"pss{g}_{kb}",
                                            tag="sc")
                            for di in range(DT):
                                nc.tensor.matmul(
                                    ps_s[:],
                                    kt_sb[:, di, kb * P:(kb + 1) * P],
                                    qt_sb[:, di, g * GQ:(g + 1) * GQ],
                                    start=(di == 0),
                                    stop=(di == DT - 1),
                                )
                            pt = ptp.tile([P, GQ], BF16, name=f"pt{g}_{kb}",
                                          tag="pt")
                            nc.scalar.activation(
                                pt[:], ps_s[:], AF.Exp, bias=0.0, scale=1.0 / 32.0
                            )
                            if kb >= MASK_START[g]:
                                keep = ptp.tile([P, GQ], BF16, name=f"kept{g}_{kb}",
                                                tag="keep")
                                if kb == 0:
                                    kp = kpos_f
                                else:
                                    kp = ptp.tile([P, 1], F32, name=f"kpt{g}_{kb}",
                                                  tag="kp")
                                    nc.vector.tensor_scalar(
                                        kp[:], kpos_f[:], float(kb * P), None,
                                        mybir.AluOpType.add,
                                    )
                                nc.vector.tensor_scalar(
                                    keep[:], qbc[:], kp[:], None,
                                    mybir.AluOpType.is_ge,
                                )
                                nc.vector.tensor_mul(pt[:], pt[:], keep[:])
                            v_sb = vin.tile([P, D], BF16, name=f"vsb_a{g}_{kb}",
                                            tag="vin")
                            nc.sync.dma_start(out=v_sb[:], in_=v_tmp[kb, :, :])
                            for tq in range(2):
                                ptq = pt[:, tq * P:(tq + 1) * P]
                                first = (kb == 0)
                                last = (kb == lkb - 1)
                                nc.tensor.matmul(
                                    oa[2 * tq][:], ptq, v_sb[:, 0:512],
                                    start=first, stop=last,
                                )
                                nc.tensor.matmul(
                                    oa[2 * tq + 1][:], ptq, v_sb[:, 512:1024],
                                    start=first, stop=last,
                                )
                                nc.tensor.matmul(
                                    sm[tq][:], ptq, ones_col[:],
                                    start=first, stop=last,
                                )

                        for tq in range(2):
                            recip = ev.tile([P, 1], F32, name=f"rc{g}_{tq}",
                                            tag="recip")
                            nc.vector.reciprocal(recip[:], sm[tq][:])
                            o_sb = ev.tile([P, D], F32, name=f"ob{g}_{tq}",
                                           tag="osb")
                            nc.scalar.mul(o_sb[:, 0:512], oa[2 * tq][:], recip[:])
                            nc.scalar.mul(
                                o_sb[:, 512:1024], oa[2 * tq + 1][:], recip[:]
                            )
                            q_local = 2 * g + tq
                            nc.sync.dma_start(
                                out=out[q_local * P:(q_local + 1) * P, :],
                                in_=o_sb[:],
                            )
    nc.compile()
    return nc


def get_nc():
    if not _NC_CACHE:
        _NC_CACHE.append(_build_nc())
    return _NC_CACHE[0]


def make_in_maps(x, Wq, Wk, Wv):
    x = np.asarray(x, dtype=np.float32)
    wqT = np.ascontiguousarray(np.asarray(Wq, np.float32).T)
    wkT = np.ascontiguousarray(np.asarray(Wk, np.float32).T)
    wvT = np.ascontiguousarray(np.asarray(Wv, np.float32).T)
    in_maps = []
    for c in range(8):
        b, h = divmod(c, 2)
        qrows = np.concatenate(
            [np.arange(qb * P, (qb + 1) * P) for qb in QLIST[h]]
        )
        xb = x[b]  # [S, D]
        in_maps.append({
            "xT": np.ascontiguousarray(xb.T),
            "xqT": np.ascontiguousarray(xb[qrows].T),
            "wqT": wqT,
            "wkT": wkT,
            "wvT": wvT,
            "qpos": qrows.astype(np.float32)[None, :],
        })
    return in_maps


def assemble_output(results):
    out = np.empty((B, S, D), dtype=np.float32)
    for c in range(8):
        b, h = divmod(c, 2)
        oc = results[c]["out"]
        for i, qb in enumerate(QLIST[h]):
            out[b, qb * P:(qb + 1) * P, :] = oc[i * P:(i + 1) * P, :]
    return out


def kernel(x, Wq, Wk, Wv):
    nc = get_nc()
    in_maps = make_in_maps(x, Wq, Wk, Wv)
    res = run_bass_kernel_spmd(nc, in_maps, core_ids=list(range(8)), trace=False)
    return assemble_output(res.results)
